# revision 64
# baseline (speedup 1.0000x reference)
"""Trainium2 Bass kernel for nn_EquivairantMultiheadAttention (sparse attention).

Contract: kernel(**inputs) takes the FULL unsharded numpy inputs (as produced by
setup_inputs()) and returns the FULL (B, N, COUT) float32 output.

Sharding: 8 cores = data-parallel over batch (2) x sequence-parallel over the
query dim n (4 slices of 512). Each core receives its batch's coset_functions
(transposed, bf16), its query-slice of pairwise_g / nbhd_idx, and all weights.

Math notes (equivalences used, all exact):
 - t3 = einsum(K, u) and every other per-(n,h) constant (b_k.Q, b_l.(Q+v))
   is constant across the softmax axis m, hence drops out of softmax.
 - e . (Q+v) = pg . G with G[n,h,:] = W_l[h-block,:]^T (Q+v)[n,h-block].
 - b_in is added after the weighted sum (weights sum to 1); the W_out
   projection runs on the PE per query block (it cannot be folded into V:
   attention weights differ per head while W_out mixes channels across heads).
 - mask is all ones (spec: fill=ones) -> masking is a no-op.

Performance structure (vs f32 baseline):
 - K and V2 rows are built once (PE matmuls) as ONE combined bf16 row
   [K(256)||V2(256)] = 1KB and gathered with a single dma_gather per chunk
   (halves gather descriptor count and HBM bytes vs 2x f32 gathers).
 - All pair-stream elementwise work is bf16 with unit innermost stride so the
   DVE 2x_1p mode applies; grouped reductions are add-trees of TensorTensor
   ops (TensorReduce has no fast mode).
 - Products and trees run IN-PLACE in the gather buffer (SBUF pressure).
 - exp and the E->[P,M,C] head-broadcast run on the idle Activation engine,
   split per m-half so their latency hides behind the other half's DVE work.
 - E stays UNNORMALIZED; out is scaled by 1/Z per head in the f32 epilogue.
   Aggregation for each m-half runs as soon as its E-expansion is ready, so
   the kvg gather buffer frees early and the in-order Pool engine can start
   the next query block's gathers while this block is still reducing.
"""

import math
import sys

import numpy as np

sys.path.insert(0, "/opt/trn_rl_repo")

B, N, M = 2, 2048, 64
C = 256  # CIN == COUT
H, D, POS = 8, 32, 6
NQ = 512  # queries per core
QB = 4  # query blocks of 128 per core
P = 128
NCORES = 8
INV_SQRT_D = 1.0 / math.sqrt(D)
NIC = 1024  # max idxs per dma_gather call (HW: larger fails the exec unit)
MH = M // 2  # m-half

_compiled = {}


def build_bass():
    import concourse.bacc as bacc
    import concourse.mybir as mybir
    import concourse.tile as tile

    dt = mybir.dt
    nc = bacc.Bacc("TRN2", target_bir_lowering=False, debug=False,
                   enable_asserts=False, num_devices=NCORES,
                   num_swdge_queues=2, dynamic_dma_scratch_size=1 << 15)

    f32 = dt.float32
    bf16 = dt.bfloat16
    i16 = dt.int16

    # ---- DRAM inputs (per core) ----
    d_cosetT = nc.dram_tensor("cosetT", [2, P, N], bf16, kind="ExternalInput")
    d_cosetQT = nc.dram_tensor("cosetQT", [2, P, NQ], bf16,
                               kind="ExternalInput")
    d_wqT = nc.dram_tensor("wqT", [2, P, C], bf16, kind="ExternalInput")
    d_wkvT = nc.dram_tensor("wkvT", [2, P, 2 * C], bf16, kind="ExternalInput")
    d_woT = nc.dram_tensor("woT", [2, P, C], bf16, kind="ExternalInput")
    d_wlBD = nc.dram_tensor("wlBD", [2, P, H * POS], bf16,
                            kind="ExternalInput")
    d_bqv = nc.dram_tensor("bqv", [2, P, 1], f32, kind="ExternalInput")
    d_bqmat = nc.dram_tensor("bqmat", [P, C], f32, kind="ExternalInput")
    d_binmat = nc.dram_tensor("binmat", [P, C], f32, kind="ExternalInput")
    d_boutmat = nc.dram_tensor("boutmat", [P, C], f32, kind="ExternalInput")
    # wrapped int16 index lists (m-major: list pos i = m*128 + n_sub)
    d_idxw = nc.dram_tensor("idxw", [P, QB, M * P // 16], i16,
                            kind="ExternalInput")
    d_pgidxw = nc.dram_tensor("pgidxw", [P, QB, M * P // 16], i16,
                              kind="ExternalInput")
    # parity masks: pgmask[k-1][p, qb, m] = 1.0 if (flatidx & 7) == k
    d_pgmask = nc.dram_tensor("pgmask", [P, 7, QB, M], dt.uint8,
                              kind="ExternalInput")
    # pairwise_g packed: row r holds flat rows 8r..8r+7, each padded 6->8
    d_pgpack = nc.dram_tensor("pgpack", [NQ * N // 8, 64], f32,
                              kind="ExternalInput")
    d_out = nc.dram_tensor("out", [QB, P, C], f32, kind="ExternalOutput")

    add = mybir.AluOpType.add
    mult = mybir.AluOpType.mult

    with tile.TileContext(nc) as tc:
        with (
            tc.tile_pool(name="const", bufs=1) as constp,
            tc.tile_pool(name="dram", bufs=1, space="DRAM") as dramp,
            tc.tile_pool(name="evac", bufs=2) as evacp,
        ):
            # index lists first: the qb0 pairwise_g gather waits only on these
            idxw = constp.tile([P, QB, M * P // 16], i16)
            pgidxw = constp.tile([P, QB, M * P // 16], i16)
            pgmask = constp.tile([P, 7, QB, M], dt.uint8)
            nc.sync.dma_start(pgidxw[:], d_pgidxw.ap())
            nc.sync.dma_start(idxw[:], d_idxw.ap())
            nc.sync.dma_start(pgmask[:], d_pgmask.ap())
            wqT = constp.tile([P, 2, C], bf16)
            wkvT = constp.tile([P, 2, 2 * C], bf16)
            woT = constp.tile([P, 2, C], bf16)
            wlBD = constp.tile([P, 2, H * POS], bf16)
            for cc in range(2):
                nc.sync.dma_start(wqT[:, cc, :], d_wqT[cc])
                nc.sync.dma_start(wkvT[:, cc, :], d_wkvT[cc])
                nc.sync.dma_start(woT[:, cc, :], d_woT[cc])
                nc.sync.dma_start(wlBD[:, cc, :], d_wlBD[cc])
            bqv = constp.tile([P, 2, 1], f32)
            nc.sync.dma_start(bqv[:], d_bqv.ap().rearrange("c p one -> p c one"))
            bqmat = constp.tile([P, C], f32)
            binmat = constp.tile([P, C], f32)
            boutmat = constp.tile([P, C], f32)
            nc.sync.dma_start(bqmat[:], d_bqmat.ap())
            nc.sync.dma_start(binmat[:], d_binmat.ap())
            nc.sync.dma_start(boutmat[:], d_boutmat.ap())
            identb = constp.tile([P, P], bf16)
            from concourse.masks import make_identity
            make_identity(nc, identb[:])

            # combined K||V2 rows in DRAM scratch (bf16, 1KB rows)
            kvdr = dramp.tile([N, 2 * C], bf16)

            q_rows = constp.tile([P, QB, C], bf16)
            g_rows = constp.tile([P, QB, H * POS], f32)

            with (
                tc.tile_pool(name="prep", bufs=1) as prepp,
                tc.tile_pool(name="psumP", bufs=2, space="PSUM") as psump,
            ):
                cosT = prepp.tile([P, 2, N], bf16)
                cosQT = prepp.tile([P, 2, NQ], bf16)
                for cc in range(2):
                    nc.sync.dma_start(cosT[:, cc, :], d_cosetT[cc])
                    nc.sync.dma_start(cosQT[:, cc, :], d_cosetQT[cc])

                # K||V build first: the gathers gate on the full kvdr table
                for jt in range(N // P):
                    ps = psump.tile([P, 2 * C], f32, tag="ps")
                    for cc in range(2):
                        nc.tensor.matmul(ps[:],
                                         lhsT=cosT[:, cc, jt * P:(jt + 1) * P],
                                         rhs=wkvT[:, cc, :],
                                         start=(cc == 0), stop=(cc == 1))
                    kv_sb = evacp.tile([P, 2 * C], bf16, tag="kvevac")
                    nc.scalar.copy(kv_sb[:], ps[:])
                    nc.sync.dma_start(kvdr[jt * P:(jt + 1) * P, :], kv_sb[:])

                # Q rows (bf16, b_q added)
                for nt in range(QB):
                    ps = psump.tile([P, C], f32, tag="ps2")
                    for cc in range(2):
                        nc.tensor.matmul(ps[:],
                                         lhsT=cosQT[:, cc, nt * P:(nt + 1) * P],
                                         rhs=wqT[:, cc, :],
                                         start=(cc == 0), stop=(cc == 1))
                    nc.vector.tensor_tensor(out=q_rows[:, nt, :], in0=ps[:],
                                            in1=bqmat[:], op=add)

                # G rows: g = (Q + v) @ W_l (block-diagonal per head)
                qvT = prepp.tile([P, 2, NQ], bf16)
                for cc2 in range(2):
                    ps = psump.tile([P, NQ], f32, tag="ps3")
                    for cc in range(2):
                        nc.tensor.matmul(ps[:],
                                         lhsT=wqT[:, cc, cc2 * P:(cc2 + 1) * P],
                                         rhs=cosQT[:, cc, :],
                                         start=(cc == 0), stop=(cc == 1))
                    nc.vector.tensor_tensor(
                        out=qvT[:, cc2, :], in0=ps[:],
                        in1=bqv[:, cc2, :].broadcast_to([P, NQ]), op=add)
                for nt in range(QB):
                    ps = psump.tile([P, H * POS], f32, tag="ps4")
                    for cc in range(2):
                        nc.tensor.matmul(ps[:],
                                         lhsT=qvT[:, cc, nt * P:(nt + 1) * P],
                                         rhs=wlBD[:, cc, :],
                                         start=(cc == 0), stop=(cc == 1))
                    nc.scalar.copy(g_rows[:, nt, :], ps[:])

            # ---- main loop: per query block, per m-half ----
            with (
                tc.tile_pool(name="gath", bufs=3) as gathp,
                tc.tile_pool(name="pgp", bufs=2) as pgp,
                tc.tile_pool(name="t2p", bufs=1) as t2p,
                tc.tile_pool(name="e3p", bufs=2) as e3p,
                tc.tile_pool(name="small", bufs=2) as smallp,
                tc.tile_pool(name="epi", bufs=1) as epip,
                tc.tile_pool(name="psumM", bufs=2, space="PSUM") as psump,
            ):
              def issue_pg_gather(qb):
                pgg = pgp.tile([P, M, 64], f32, tag="pgg")
                for k in range(M * P // NIC):
                    nc.gpsimd.dma_gather(
                        out_ap=pgg[:, k * (NIC // P):(k + 1) * (NIC // P), :],
                        in_ap=d_pgpack.ap()[qb * (P * N // 8):
                                            (qb + 1) * (P * N // 8), :],
                        idxs_ap=idxw_slice(pgidxw, qb, k, NIC),
                        num_idxs=NIC, num_idxs_reg=NIC, elem_size=64,
                        queue_num=0)
                return pgg

              # one block of pairwise_g prefetched; later blocks issue after
              # each block's KV gathers so they never delay the critical kv0
              pgg_fifo = [issue_pg_gather(0)]
              for qb in range(QB):
                # --- parity select + t2 -> A (gather issued ahead) ---
                pgg = pgg_fifo.pop(0)
                # parity select in place: target cols 0:6 never source k>=1
                pg6 = pgg[:, :, 0:POS]
                for k in range(1, 8):
                    nc.vector.copy_predicated(
                        out=pg6,
                        mask=pgmask[:, k - 1, qb, :][:, :, None]
                            .broadcast_to([P, M, POS]),
                        data=pgg[:, :, 8 * k:8 * k + POS])

                A = smallp.tile([P, M, H], f32, tag="A")

                # --- KV gathers per m-half (issued up front) ---
                kvgs = []
                for mh in range(2):
                    kvg = gathp.tile([P, MH, 2 * C], bf16, tag="kvg")
                    kvgs.append(kvg)
                    for k in range(MH * P // NIC):
                        nc.gpsimd.dma_gather(
                            out_ap=kvg[:, k * (NIC // P):(k + 1) * (NIC // P),
                                       :],
                            in_ap=kvdr[:],
                            idxs_ap=idxw_slice(idxw, qb,
                                               mh * (MH * P // NIC) + k, NIC),
                            num_idxs=NIC, num_idxs_reg=NIC, elem_size=2 * C,
                            queue_num=0)
                # prefetch the next block's pairwise_g behind this block's KVs
                if qb + 1 < QB:
                    pgg_fifo.append(issue_pg_gather(qb + 1))

                # --- t2 product + reduce on the DVE (the Pool
                # scalar_tensor_tensor path fails neuronx-cc compilation) ---
                for mh in range(2):
                    ms = slice(mh * MH, (mh + 1) * MH)
                    t2t = t2p.tile([P, MH, H, POS], f32, tag="t2t")
                    g_bc = (g_rows[:, qb, :]
                            .rearrange("p (h pp) -> p h pp", pp=POS)
                            [:, None, :, :].broadcast_to([P, MH, H, POS]))
                    pg_bc = (pgg[:, ms, None, 0:POS]
                             .broadcast_to([P, MH, H, POS]))
                    nc.vector.tensor_tensor(out=t2t[:], in0=pg_bc,
                                            in1=g_bc, op=mult)
                    nc.vector.tensor_reduce(
                        out=A[:, ms, :].rearrange("p m h -> p (m h)"),
                        in_=t2t[:].rearrange("p m h pp -> p (m h) pp"),
                        axis=mybir.AxisListType.X, op=add)

                # --- t1 scores + exp per m-half (Act runs ahead) ---
                E = smallp.tile([P, M, H], bf16, tag="E")
                e3s = []
                for mh in range(2):
                    kvg = kvgs[mh]
                    ms = slice(mh * MH, (mh + 1) * MH)
                    # t1 product in-place on the K half (bf16 2x mode)
                    kh = kvg[:, :, 0:C]
                    nc.vector.tensor_tensor(
                        out=kh, in0=kh,
                        in1=q_rows[:, qb, :][:, None, :].broadcast_to(
                            [P, MH, C]),
                        op=mult)
                    # d-tree: sum groups of 32 (head dim), in-place to 1 col
                    kh4 = kvg[:, :, 0:C].rearrange("p m (h d) -> p m h d", d=D)
                    half = D // 2
                    while half >= 1:
                        nc.vector.tensor_tensor(
                            out=kh4[:, :, :, 0:half],
                            in0=kh4[:, :, :, 0:half],
                            in1=kh4[:, :, :, half:2 * half], op=add)
                        half //= 2
                    # A[:, mh-slice, :] += t1 (strided bf16 view into f32 A)
                    nc.vector.tensor_tensor(
                        out=A[:, ms, :], in0=A[:, ms, :],
                        in1=kh4[:, :, :, 0], op=add)
                    # exp of this half (Act); E stays unnormalized
                    nc.scalar.activation(
                        out=E[:, ms, :], in_=A[:, ms, :],
                        func=mybir.ActivationFunctionType.Exp,
                        scale=INV_SQRT_D)
                    # expand E -> [P, MQ, C] per quarter on Act (smaller
                    # tiles double-buffer within the half)
                    MQ = MH // 2
                    for q in range(2):
                        e3 = e3p.tile([P, MQ, C], bf16, tag="e3")
                        qs_ = slice(mh * MH + q * MQ, mh * MH + (q + 1) * MQ)
                        nc.scalar.copy(
                            e3[:].rearrange("p m (h d) -> p m h d", d=D),
                            E[:, qs_, :, None].broadcast_to([P, MQ, H, D]))
                        e3s.append(e3)

                # --- aggregation per m-quarter into the half tiles ---
                MQ = MH // 2
                oa = epip.tile([P, 4, C], f32, tag="oa")
                for mq in range(4):
                    kvg = kvgs[mq // 2]
                    vh = kvg[:, (mq % 2) * MQ:(mq % 2 + 1) * MQ, C:2 * C]
                    nc.vector.tensor_tensor(out=vh, in0=vh, in1=e3s[mq][:],
                                            op=mult)
                    half = MQ // 2
                    while half >= 2:
                        nc.vector.tensor_tensor(
                            out=vh[:, 0:half, :], in0=vh[:, 0:half, :],
                            in1=vh[:, half:2 * half, :], op=add)
                        half //= 2
                    nc.vector.tensor_tensor(out=oa[:, mq, :],
                                            in0=vh[:, 0, :], in1=vh[:, 1, :],
                                            op=add)

                # --- softmax denominator: one strided reduce over m ---
                z = epip.tile([P, H], f32, tag="z")
                nc.vector.tensor_reduce(
                    out=z[:], in_=E[:].transpose([0, 2, 1]),
                    axis=mybir.AxisListType.X, op=add)
                rz = epip.tile([P, H], f32, tag="rz")
                nc.vector.reciprocal(rz[:], z[:])

                # --- epilogue: combine quarters, scale by 1/Z, + b_in ---
                nc.vector.tensor_tensor(out=oa[:, 0, :], in0=oa[:, 0, :],
                                        in1=oa[:, 1, :], op=add)
                nc.vector.tensor_tensor(out=oa[:, 2, :], in0=oa[:, 2, :],
                                        in1=oa[:, 3, :], op=add)
                agg = epip.tile([P, C], f32, tag="agg")
                nc.vector.tensor_tensor(out=agg[:], in0=oa[:, 0, :],
                                        in1=oa[:, 2, :], op=add)
                agv = agg[:].rearrange("p (h d) -> p h d", d=D)
                nc.vector.tensor_tensor(
                    out=agv, in0=agv,
                    in1=rz[:, :, None].broadcast_to([P, H, D]), op=mult)
                aggb = epip.tile([P, C], bf16, tag="aggb")
                nc.vector.tensor_tensor(out=aggb[:], in0=agg[:],
                                        in1=binmat[:], op=add)

                # --- output projection: out = (agg+b_in) @ W_out^T + b_out ---
                aggT = epip.tile([P, 2, P], bf16, tag="aggT")
                for cc in range(2):
                    pst = psump.tile([P, P], bf16, tag="pst")
                    nc.tensor.transpose(pst[:], aggb[:, cc * P:(cc + 1) * P],
                                        identb[:])
                    nc.scalar.copy(aggT[:, cc, :], pst[:])
                psO = psump.tile([P, C], f32, tag="psO")
                for cc in range(2):
                    nc.tensor.matmul(psO[:], lhsT=aggT[:, cc, :],
                                     rhs=woT[:, cc, :],
                                     start=(cc == 0), stop=(cc == 1))
                out_sb = epip.tile([P, C], f32, tag="outsb")
                nc.vector.tensor_tensor(out=out_sb[:], in0=psO[:],
                                        in1=boutmat[:], op=add)
                nc.sync.dma_start(d_out[qb], out_sb[:])

    nc.compile()
    return nc


def idxw_slice(idxw_tile, qb, k, nic):
    """Column slice of the wrapped idx tile for gather chunk k (nic idxs)."""
    ncols = nic // 16
    return idxw_tile[:, qb, k * ncols:(k + 1) * ncols]


def _wrap_idx(lst):
    """int16 list -> [128, len/16] wrapped (pos i -> [i%16, i//16]) and
    replicated across the 8 groups of 16 partitions."""
    n = lst.shape[0]
    w = np.empty((P, n // 16), np.int16)
    blk = lst.reshape(n // 16, 16).T  # [16, n/16]
    for g in range(8):
        w[g * 16:(g + 1) * 16, :] = blk
    return w


def _bf16(x):
    import ml_dtypes
    return np.ascontiguousarray(np.asarray(x, np.float32)
                                .astype(ml_dtypes.bfloat16))


def make_core_inputs(pairwise_g, coset_functions, nbhd_idx,
                     W_q, b_q, W_k, W_l, u, v, W_in, b_in, W_out, b_out):
    pairwise_g = np.asarray(pairwise_g)
    coset_functions = np.asarray(coset_functions)
    nbhd_idx = np.asarray(nbhd_idx)
    W_q = np.asarray(W_q, np.float32)
    W_k = np.asarray(W_k, np.float32)
    W_l = np.asarray(W_l, np.float32)
    W_in = np.asarray(W_in, np.float32)
    W_out = np.asarray(W_out, np.float32)
    b_q = np.asarray(b_q, np.float32)
    b_in = np.asarray(b_in, np.float32)
    b_out = np.asarray(b_out, np.float32)
    v = np.asarray(v, np.float32)

    wqT = _bf16(W_q.T.reshape(2, P, C))
    # combined K || V rows (V = W_in projection; W_out applied on device)
    wkvT = _bf16(np.concatenate([W_k.T, W_in.T], axis=1).reshape(2, P, 2 * C))
    woT = _bf16(W_out.T.reshape(2, P, C))
    wlBD_full = np.zeros((C, H * POS), np.float32)
    for h in range(H):
        wlBD_full[h * D:(h + 1) * D, h * POS:(h + 1) * POS] = \
            W_l[h * D:(h + 1) * D, :]
    wlBD = _bf16(wlBD_full.reshape(2, P, H * POS))
    bqv = np.ascontiguousarray(
        (b_q + v.reshape(C)).reshape(2, P, 1).astype(np.float32))
    bqmat = np.ascontiguousarray(np.broadcast_to(b_q, (P, C)))
    binmat = np.ascontiguousarray(np.broadcast_to(b_in, (P, C)))
    boutmat = np.ascontiguousarray(np.broadcast_to(b_out, (P, C)))

    in_maps = []
    for core in range(NCORES):
        b = core // 4
        qs = (core % 4) * NQ
        cosetT = _bf16(coset_functions[b].T.reshape(2, P, N))
        cosetQT = _bf16(coset_functions[b, qs:qs + NQ].T.reshape(2, P, NQ))
        idx = nbhd_idx[b, qs:qs + NQ].astype(np.int64)  # [NQ, M]

        idxw = np.empty((P, QB, M * P // 16), np.int16)
        pgidxw = np.empty((P, QB, M * P // 16), np.int16)
        pgmask = np.zeros((P, 7, QB, M), np.uint8)
        for qb in range(QB):
            blk = idx[qb * P:(qb + 1) * P]  # [P(n), M]
            # m-major list: pos i = m*128 + n
            lst = blk.T.reshape(M * P)  # [m, n] flattened
            idxw[:, qb, :] = _wrap_idx(lst.astype(np.int16))
            flat = (np.arange(P, dtype=np.int64)[None, :] * N
                    + blk.T)  # [m, n] local flat
            pgidxw[:, qb, :] = _wrap_idx(
                (flat.reshape(M * P) >> 3).astype(np.int16))
            par = (blk & 7)  # [P(n), M] (n*N is a multiple of 8)
            for k in range(1, 8):
                pgmask[:, k - 1, qb, :] = (par == k).astype(np.uint8)

        # packed pairwise_g: row r = flat rows 8r..8r+7, padded 6->8 floats
        pgs = pairwise_g[b, qs:qs + NQ].reshape(NQ * N, POS).astype(np.float32)
        pgpack = np.zeros((NQ * N // 8, 8, 8), np.float32)
        pgpack[:, :, 0:POS] = pgs.reshape(NQ * N // 8, 8, POS)
        pgpack = np.ascontiguousarray(pgpack.reshape(NQ * N // 8, 64))

        in_maps.append({
            "cosetT": cosetT, "cosetQT": cosetQT,
            "wqT": wqT, "wkvT": wkvT, "woT": woT,
            "wlBD": wlBD, "bqv": bqv, "bqmat": bqmat,
            "binmat": binmat, "boutmat": boutmat,
            "idxw": idxw, "pgidxw": pgidxw, "pgmask": pgmask,
            "pgpack": pgpack,
        })
    return in_maps


def assemble_output(results):
    out = np.empty((B, N, C), np.float32)
    for core in range(NCORES):
        b = core // 4
        qs = (core % 4) * NQ
        o = results[core]["out"]  # [QB, P, C]
        out[b, qs:qs + NQ] = o.reshape(NQ, C)
    return out


def kernel(pairwise_g, coset_functions, mask, nbhd_idx,
           W_q, b_q, W_k, b_k, W_l, b_l, u, v,
           W_in, b_in, W_out, b_out, **_unused):
    from concourse.bass_utils import run_bass_kernel_spmd

    if "nc" not in _compiled:
        _compiled["nc"] = build_bass()
    nc = _compiled["nc"]

    in_maps = make_core_inputs(pairwise_g, coset_functions, nbhd_idx,
                               W_q, b_q, W_k, W_l, u, v, W_in, b_in,
                               W_out, b_out)
    res = run_bass_kernel_spmd(nc, in_maps, core_ids=list(range(NCORES)))
    return assemble_output(res.results)
